# revision 1
# baseline (speedup 1.0000x reference)
"""Minibatch K-means one-step on 8 Trainium2 NeuronCores (Bass/Tile).

Algorithm (matches reference):
  distance[n,k] = ||s_n||^2 + ||m_k||^2 - 2 s.m
  bins = argmin_k distance ; inertia = sum sqrt(min distance)
  counts = bincount(bins) ; cluster_sums[k] = sum_{bins(n)=k} s_n
  new_means = (cluster_sums + means*weight_sum) / max(weight_sum+counts, 1)
  keep old mean where counts==0

Sharding: data-parallel over n_samples across 8 cores. Each core:
  phase 1: psum[n,k] = s.m - via bf16 hi/lo split (3 matmuls: hh, hl, lh)
           -> fp32-grade precision at 3 cyc/row instead of fp32's 4.
           negdist = 2*psum - ||m||^2 (DVE), argmax via max/max_index.
  phase 2: one-hot A[n,k] = (iota==bin) [exact in bf16]; cluster_sums =
           A^T @ (s_hi + s_lo) via 2 exact bf16 matmuls into psum.
Host: shard/split inputs, bincount bins, all-reduce partials, final
EMA mean update (tiny O(K*D) elementwise, mirrors reference ops in fp32).
"""
import sys
import numpy as np

sys.path.insert(0, "/opt/trn_rl_repo")

import ml_dtypes
import concourse.bass as bass  # noqa: F401  (AP types)
import concourse.bacc as bacc
import concourse.tile as tile
from contextlib import ExitStack
from concourse import mybir
from concourse.bass_utils import run_bass_kernel_spmd

F32 = mybir.dt.float32
BF16 = mybir.dt.bfloat16
U32 = mybir.dt.uint32
BF16_NP = ml_dtypes.bfloat16

G, N, K, D = 1, 32768, 2048, 512
M = 8                    # cores
NL = N // M              # 4096 samples per core
NCH = NL // 128          # 32 n-chunks of 128 samples
DC = D // 128            # 4 contraction chunks
KC = K // 512            # 4 k-chunks of 512 (phase 1)
KP = K // 128            # 16 k-chunks of 128 (phase 2)
AluOp = mybir.AluOpType
ActFn = mybir.ActivationFunctionType

_CACHE = {}


def build_program():
    nc = bacc.Bacc("TRN2", target_bir_lowering=False, debug=False,
                   enable_asserts=True, num_devices=M)

    # -------- DRAM I/O (per core) --------
    sT_hi = nc.dram_tensor("sT_hi", [NCH, DC, 128, 128], BF16, kind="ExternalInput").ap()
    sT_lo = nc.dram_tensor("sT_lo", [NCH, DC, 128, 128], BF16, kind="ExternalInput").ap()
    s_hi = nc.dram_tensor("s_hi", [NL, D], BF16, kind="ExternalInput").ap()
    s_lo = nc.dram_tensor("s_lo", [NL, D], BF16, kind="ExternalInput").ap()
    mT_hi = nc.dram_tensor("mT_hi", [DC, 128, K], BF16, kind="ExternalInput").ap()
    mT_lo = nc.dram_tensor("mT_lo", [DC, 128, K], BF16, kind="ExternalInput").ap()
    m2_in = nc.dram_tensor("m2", [128, K], F32, kind="ExternalInput").ap()
    iota_in = nc.dram_tensor("iota", [128, K], F32, kind="ExternalInput").ap()

    csums_out = nc.dram_tensor("csums", [KP, 128, D], F32, kind="ExternalOutput").ap()
    bins_out = nc.dram_tensor("bins", [128, NCH], U32, kind="ExternalOutput").ap()
    sqrt_out = nc.dram_tensor("sqrtsel", [128, NCH], F32, kind="ExternalOutput").ap()

    with tile.TileContext(nc) as tc:
        with ExitStack() as ctx:
            const = ctx.enter_context(tc.tile_pool(name="const", bufs=1))
            small = ctx.enter_context(tc.tile_pool(name="small", bufs=1))

            # ---- resident constants ----
            m2 = const.tile([128, K], F32, name="m2t")
            nc.sync.dma_start(m2[:], m2_in)
            iota = const.tile([128, K], F32, name="iotat")
            nc.sync.dma_start(iota[:], iota_in)
            mh = []
            ml = []
            for d in range(DC):
                t = const.tile([128, K], BF16, name=f"mh{d}")
                nc.sync.dma_start(t[:], mT_hi[d])
                mh.append(t)
                t = const.tile([128, K], BF16, name=f"ml{d}")
                nc.sync.dma_start(t[:], mT_lo[d])
                ml.append(t)

            # ---- resident per-sample scalars ----
            mxall = small.tile([128, NCH], F32, name="mxall")
            binsf = small.tile([128, NCH], F32, name="binsf")
            binsu = small.tile([128, NCH], U32, name="binsu")
            s2all = small.tile([128, NCH], F32, name="s2all")

            # ================= phase 1: distances + argmin =================
            with ExitStack() as p1:
                sbuf = p1.enter_context(tc.tile_pool(name="p1sb", bufs=3))
                nds = p1.enter_context(tc.tile_pool(name="p1nd", bufs=2))
                mxp = p1.enter_context(tc.tile_pool(name="p1mx", bufs=4))
                ps1 = p1.enter_context(tc.tile_pool(name="p1ps", bufs=2, space="PSUM"))

                for c in range(NCH):
                    sh = sbuf.tile([128, DC, 128], BF16, tag="sh", name=f"sh{c}")
                    nc.sync.dma_start(sh[:], sT_hi[c].rearrange("d p n -> p d n"))
                    sl = sbuf.tile([128, DC, 128], BF16, tag="sl", name=f"sl{c}")
                    nc.sync.dma_start(sl[:], sT_lo[c].rearrange("d p n -> p d n"))

                    ps = [ps1.tile([128, 512], F32, tag=f"ps{j}", name=f"ps{j}_{c}")
                          for j in range(KC)]
                    # accumulation: hh + hl + lh per (d-chunk), weights reused
                    # across the 4 k-chunks of 512
                    nmm = 3 * DC
                    i = 0
                    for d in range(DC):
                        for lhsT, rhs in ((sh[:, d], mh[d]), (sh[:, d], ml[d]),
                                          (sl[:, d], mh[d])):
                            for j in range(KC):
                                nc.tensor.matmul(
                                    ps[j][:], lhsT=lhsT, rhs=rhs[:, j * 512:(j + 1) * 512],
                                    start=(i == 0), stop=(i == nmm - 1))
                            i += 1
                    nd = nds.tile([128, K], F32, tag="nd", name=f"nd{c}")
                    for j in range(KC):
                        js = slice(j * 512, (j + 1) * 512)
                        # nd = 2*psum - m2   (argmax nd == argmin distance)
                        nc.vector.scalar_tensor_tensor(
                            nd[:, js], in0=ps[j][:], scalar=2.0, in1=m2[:, js],
                            op0=AluOp.mult, op1=AluOp.subtract)
                    mx8 = mxp.tile([128, 8], F32, tag="mx8", name=f"mx8_{c}")
                    mi8 = mxp.tile([128, 8], U32, tag="mi8", name=f"mi8_{c}")
                    nc.vector.max(mx8[:], nd[:])
                    nc.vector.max_index(mi8[:], mx8[:], nd[:])
                    nc.vector.tensor_copy(mxall[:, c:c + 1], mx8[:, 0:1])
                    nc.vector.tensor_copy(binsu[:, c:c + 1], mi8[:, 0:1])
                    nc.vector.tensor_copy(binsf[:, c:c + 1], mi8[:, 0:1])

            # ================= phase 2: scatter-add via one-hot matmul ======
            with ExitStack() as p2:
                sbuf2 = p2.enter_context(tc.tile_pool(name="p2sb", bufs=3))
                apool = p2.enter_context(tc.tile_pool(name="p2a", bufs=4))
                opool = p2.enter_context(tc.tile_pool(name="p2o", bufs=2))
                ps2 = p2.enter_context(tc.tile_pool(name="p2ps", bufs=1, space="PSUM"))

                for p in range(2):           # two passes of 8 psum banks
                    cs = [ps2.tile([128, 512], F32, tag=f"cs{j}", name=f"cs{j}_{p}")
                          for j in range(8)]
                    for c in range(NCH):
                        rh = sbuf2.tile([128, D], BF16, tag="rh", name=f"rh{p}_{c}")
                        nc.sync.dma_start(rh[:], s_hi[c * 128:(c + 1) * 128, :])
                        rl = sbuf2.tile([128, D], BF16, tag="rl", name=f"rl{p}_{c}")
                        nc.sync.dma_start(rl[:], s_lo[c * 128:(c + 1) * 128, :])

                        if p == 0:
                            # ||s||^2 from reconstructed fp32 samples (error
                            # ~2^-18 relative; only feeds inertia)
                            sf = sbuf2.tile([128, D], F32, tag="sf", name=f"sf{c}")
                            nc.vector.tensor_add(sf[:], rh[:], rl[:])
                            sq = sbuf2.tile([128, D], F32, tag="sq", name=f"sq{c}")
                            nc.scalar.activation(sq[:], sf[:], ActFn.Square,
                                                 accum_out=s2all[:, c:c + 1])

                        for j in range(8):
                            k0 = (p * 8 + j) * 128
                            a = apool.tile([128, 128], BF16, tag="a", name=f"a{p}_{c}_{j}")
                            nc.vector.tensor_scalar(
                                out=a[:], in0=iota[:, k0:k0 + 128],
                                scalar1=binsf[:, c:c + 1], scalar2=None,
                                op0=AluOp.is_equal)
                            nc.tensor.matmul(cs[j][:], lhsT=a[:], rhs=rh[:],
                                             start=(c == 0), stop=False)
                            nc.tensor.matmul(cs[j][:], lhsT=a[:], rhs=rl[:],
                                             start=False, stop=(c == NCH - 1))
                    for j in range(8):
                        o = opool.tile([128, 512], F32, tag="o", name=f"o{p}_{j}")
                        nc.vector.tensor_copy(o[:], cs[j][:])
                        nc.sync.dma_start(csums_out[p * 8 + j], o[:])

            # ================= tail: selected distance + sqrt ===============
            sel = small.tile([128, NCH], F32, name="sel")
            nc.vector.scalar_tensor_tensor(sel[:], in0=mxall[:], scalar=-1.0,
                                           in1=s2all[:], op0=AluOp.mult,
                                           op1=AluOp.add)
            sq2 = small.tile([128, NCH], F32, name="sq2")
            nc.scalar.activation(sq2[:], sel[:], ActFn.Sqrt)
            nc.sync.dma_start(sqrt_out, sq2[:])
            nc.sync.dma_start(bins_out, binsu[:])

    nc.compile()
    return nc


def _split_bf16(x):
    hi = x.astype(BF16_NP)
    lo = (x - hi.astype(np.float32)).astype(BF16_NP)
    return hi, lo


def make_in_maps(samples, means):
    """Host-side sharding + layout prep. samples [N,D] f32, means [K,D] f32."""
    mT = np.ascontiguousarray(means.T)                     # [D, K]
    mT_hi, mT_lo = _split_bf16(mT)
    mT_hi = mT_hi.reshape(DC, 128, K)
    mT_lo = mT_lo.reshape(DC, 128, K)
    m2 = (means.astype(np.float64) ** 2).sum(1).astype(np.float32)  # [K]
    m2 = np.broadcast_to(m2[None, :], (128, K)).copy()
    iota = np.broadcast_to(np.arange(K, dtype=np.float32)[None, :], (128, K)).copy()

    in_maps = []
    for i in range(M):
        s = samples[i * NL:(i + 1) * NL]                   # [NL, D] f32
        s_hi, s_lo = _split_bf16(s)
        # transposed layout, contiguous per n-chunk: [NCH, DC, 128, 128]
        sT_hi = np.ascontiguousarray(
            s_hi.T.reshape(DC, 128, NCH, 128).transpose(2, 0, 1, 3))
        sT_lo = np.ascontiguousarray(
            s_lo.T.reshape(DC, 128, NCH, 128).transpose(2, 0, 1, 3))
        in_maps.append({
            "sT_hi": sT_hi, "sT_lo": sT_lo,
            "s_hi": np.ascontiguousarray(s_hi),
            "s_lo": np.ascontiguousarray(s_lo),
            "mT_hi": mT_hi, "mT_lo": mT_lo, "m2": m2, "iota": iota,
        })
    return in_maps


def assemble(results, samples, means, weight_sum):
    """Host-side unshard: all-reduce partials + EMA mean update (fp32,
    mirroring the reference elementwise ops)."""
    csums64 = np.zeros((K, D), np.float64)
    bins_all = np.empty((N,), np.int64)
    inertia64 = 0.0
    for i, r in enumerate(results):
        csums64 += r["csums"].reshape(K, D).astype(np.float64)
        bins_all[i * NL:(i + 1) * NL] = r["bins"].T.reshape(NL).astype(np.int64)
        inertia64 += r["sqrtsel"].T.reshape(NL).astype(np.float64).sum()

    cluster_sums = csums64.astype(np.float32)              # [K, D]
    counts = np.bincount(bins_all, minlength=K).astype(np.float32)  # [K]

    ws = weight_sum[0]                                     # [K] f32
    new_means = cluster_sums + means * ws[:, None]
    new_ws = ws + counts
    alpha = (np.float32(1.0) / np.where(new_ws == 0, np.float32(1.0), new_ws)
             ).astype(np.float32)
    new_means = new_means * alpha[:, None]
    new_means = np.where((counts == 0)[:, None], means, new_means)
    inertia = np.float32(inertia64)
    return new_means[None].astype(np.float32), inertia


def kernel(samples, means, weight_sum):
    samples = np.asarray(samples, dtype=np.float32)
    means = np.asarray(means, dtype=np.float32)
    weight_sum = np.asarray(weight_sum, dtype=np.float32)

    if "nc" not in _CACHE:
        _CACHE["nc"] = build_program()
    nc = _CACHE["nc"]

    in_maps = make_in_maps(samples[0], means[0])
    res = run_bass_kernel_spmd(nc, in_maps, core_ids=list(range(M)))
    return assemble(res.results, samples[0], means[0], weight_sum)


if __name__ == "__main__":
    # smoke test against local numpy reference
    rng = np.random.default_rng(0)
    samples = rng.standard_normal((G, N, D), dtype=np.float32)
    means = rng.standard_normal((G, K, D), dtype=np.float32)
    weight_sum = np.zeros((G, K), np.float32)
    out, inertia = kernel(samples=samples, means=means, weight_sum=weight_sum)
    print(out.shape, inertia)
